# revision 8
# baseline (speedup 1.0000x reference)
"""GCN (2-layer) + mean-pool + MLP head on 8 TRN2 NeuronCores.

Strategy (dst-sharded graph partitioning, band-pipelined):
- Nodes sharded 8 ways; core c owns nodes [c*12500, (c+1)*12500) and all edges
  whose dst lands in its shard. Within a core, nodes are placed into 4 bands
  of 25 chunks x 128 slots (3200 slots, 3125 real nodes per band); a greedy
  balancer permutes nodes within their band so per-(src-band, dst-chunk) edge
  counts match across cores (minimizes padding of the shared stream layouts).
- GCN norm factorizes: out[v] = dis[v]*(sum_{u->v} dis[u]*h[u] + dis[v]*h[v]).
  Host scales x rows by dis[src] and bakes pure-0/1 one-hot scatter tiles in
  fp8 (exact); dis[v] is applied on device as a per-partition scalar when the
  aggregate leaves the accumulator. Self-loops ride the L1 stream as extra
  edges; L2 self-loops reuse the SBUF-resident relu1 tiles via an identity
  matmul (table rows already carry dis[v]*relu1[v]).
- Layer 1 streams dense per-edge messages (fp8 dis[src]*x[src], chunk-major
  sorted, padded) host-side; PE contracts fp8 0/1 one-hots into per-chunk
  PSUM chains drained to an f32 band accumulator.
- Band pipeline: for band b: L1 stream calls -> 25 chunk transforms (scale,
  transpose, @W1+b1, relu with dis scale -> resident rel1 + agin DMA + self
  add into acc2) -> AllGather(b) -> L2 group b gathers (dma_gather from the
  AllGathered table, SWDGE queues rotated) + fp8 one-hot matmuls adding into
  acc2. Gathers/matmuls of group b overlap the L1 streaming of band b+1.
- Phase 3: per chunk scale/transpose/@W2+b2/relu/pool matmul into per-core
  partials (no DMA); AllReduce; MLP head computed redundantly; core 0 output.
"""
import sys
sys.path.insert(0, '/opt/trn_rl_repo')
import contextlib
import numpy as np
import ml_dtypes

import concourse.bass as bass
import concourse.bacc as bacc
import concourse.mybir as mybir
import concourse.tile as tile
from concourse import library_config
from concourse.bass_utils import run_bass_kernel_spmd

BF16 = ml_dtypes.bfloat16
FP8 = ml_dtypes.float8_e4m3
CORES = 8
F = 128          # feature/hidden width (fixed at 128 = partition width)
NGRP = 4         # src bands / gather groups (int16 gather index limit)
CHB = 25         # chunks per band
CT1 = 32         # tiles per L1 stream call
CT2 = 48         # tiles per gather call
NQ = 4           # SWDGE queues rotated across gather calls


class Geom:
    def __init__(self, n_nodes=100000, n_edges=1600000, n_graphs=64, a_dim=8):
        self.N = n_nodes
        self.E = n_edges
        self.G = n_graphs
        self.A = a_dim
        self.NLOC = n_nodes // CORES          # 12500
        self.BND = self.NLOC // NGRP          # 3125 real nodes per band
        self.BSLOT = CHB * 128                # 3200 slots per band
        self.CH = NGRP * CHB                  # 100 chunks per core
        self.GRP2 = CORES * self.BSLOT        # 25600 gather rows per group
        assert self.GRP2 <= 32767, "int16 gather index limit"


def _piece_plan(seg_counts_max, n_segs, call_tiles, seg_grp=None, ngrp=1):
    """Build padded stream layout, piece list and call plan.

    seg_counts_max: [n_segs] padded length per segment (max across cores).
    seg_grp: segment -> stream group (streams padded to x128 per group);
      None = single group.
    """
    if seg_grp is None:
        seg_grp = np.zeros(n_segs, np.int64)
    base = np.zeros(n_segs + 1, np.int64)
    grp_len = [0] * ngrp
    grp_lo = [0] * ngrp
    off = 0
    for g in range(ngrp):
        grp_lo[g] = off
        for s in range(n_segs):
            if seg_grp[s] != g:
                continue
            base[s] = off
            off += int(seg_counts_max[s])
        if off % 128:
            off += 128 - off % 128
        grp_len[g] = off - grp_lo[g]
    base[-1] = off
    grp_tiles = [gl // 128 for gl in grp_len]

    piece_tile, piece_seg, piece_first, piece_last = [], [], [], []
    pieces_by_grp = []
    for g in range(ngrp):
        plist = []
        for s in range(n_segs):
            if seg_grp[s] != g:
                continue
            lo = int(base[s]) - grp_lo[g]
            hi = lo + int(seg_counts_max[s])
            if hi == lo:
                continue
            tlo, thi = lo // 128, (hi - 1) // 128
            for t in range(tlo, thi + 1):
                plist.append((t, s, t == tlo, t == thi))
        pieces_by_grp.append(plist)
        for (t, s, fi, la) in plist:
            piece_tile.append(t)
            piece_seg.append(s)
            piece_first.append(fi)
            piece_last.append(la)

    call_plan = []
    for g in range(ngrp):
        plist = pieces_by_grp[g]
        calls = []
        t0 = 0
        pi = 0
        left = grp_tiles[g]
        while left > 0:
            take = min(call_tiles, left)
            np_call = 0
            while pi < len(plist) and plist[pi][0] < t0 + take:
                np_call += 1
                pi += 1
            calls.append((take, np_call))
            t0 += take
            left -= take
        assert pi == len(plist)
        call_plan.append(calls)

    pk = {}
    p = 0
    for g in range(ngrp):
        for (t, s, fi, la) in pieces_by_grp[g]:
            pk[(g, t, s)] = p
            p += 1
    return dict(base=base, S_total=off, grp_tiles=grp_tiles, grp_lo=grp_lo,
                piece_tile=piece_tile, piece_seg=piece_seg,
                piece_first=piece_first, piece_last=piece_last,
                call_plan=call_plan, pk=pk, NP=len(piece_tile))


def _onehot_tiles(npieces, pos, grp_of_e, grp_lo, pk_lookup, sl):
    """A[piece, slot(=pos%128), dst_slot] = 1, flattened to [128, NP*128],
    fp8 (exact 0/1)."""
    A = np.zeros(npieces * 128 * 128, FP8)
    e_slot = np.empty(len(pos), np.int64)
    for g in np.unique(grp_of_e):
        m = grp_of_e == g
        e_slot[m] = (pos[m] - grp_lo[g]) % 128
    A[pk_lookup * (128 * 128) + e_slot * 128 + sl] = np.float32(1.0)
    return np.ascontiguousarray(
        A.reshape(npieces, 128, 128).transpose(1, 0, 2).reshape(128, npieces * 128))


def _prep(geom, x, edge_index, batch, W1, b1, W2, b2, fc1_w, fc1_b, fc2_w, fc2_b):
    """Host-side preprocessing: degrees, node placement, edge sharding/sorting,
    padding plan, per-core input arrays."""
    g_ = geom
    N, NLOC, BND, BSLOT, CH, GRP2 = g_.N, g_.NLOC, g_.BND, g_.BSLOT, g_.CH, g_.GRP2
    src = np.asarray(edge_index[0], dtype=np.int64)
    dst = np.asarray(edge_index[1], dtype=np.int64)
    batch = np.asarray(batch, dtype=np.int64)

    deg = np.bincount(dst, minlength=N).astype(np.float32) + 1.0
    dis = (1.0 / np.sqrt(deg)).astype(np.float32)

    u = np.arange(N, dtype=np.int64)
    core_of_node = u // NLOC
    band_of_node = (u % NLOC) // BND          # fixed band assignment

    core_of = dst // NLOC

    # --- greedy placement: per core, per band, assign nodes to band slots so
    # per-(src-band, dst-chunk) in-edge counts balance across cores ---
    slot_of_node = np.empty(N, np.int64)      # slot within band [0, BSLOT)
    for c in range(CORES):
        m = core_of == c
        d_loc = dst[m] - c * NLOC
        sb = band_of_node[src[m]]
        dvec = np.zeros((NLOC, NGRP), np.int64)
        np.add.at(dvec, (d_loc, sb), 1)
        for b in range(NGRP):
            nodes = np.arange(b * BND, (b + 1) * BND)  # local indices
            nodes = nodes[np.argsort(-dvec[nodes].sum(1), kind='stable')]
            Lb = np.zeros((CHB, NGRP), np.float64)
            nxt = np.arange(CHB) * 128
            left = np.full(CHB, 128, np.int64)
            pos = np.empty(len(nodes), np.int64)
            for i, v in enumerate(nodes):
                dots = Lb @ dvec[v]
                dots[left <= 0] = np.inf
                j = int(np.argmin(dots))
                pos[i] = nxt[j]
                nxt[j] += 1
                left[j] -= 1
                Lb[j] += dvec[v]
            slot_of_node[c * NLOC + nodes] = pos

    # table row within gather group = core*BSLOT + slot
    row_of_node = core_of_node * BSLOT + slot_of_node
    # global stream-table row (for xs1 gather)
    grow_of_node = band_of_node * (CORES * BSLOT) + row_of_node
    # per-core position = band*BSLOT + slot  (chunk = pos // 128)
    pos_of_node = band_of_node * BSLOT + slot_of_node

    # dis[src]-scaled node features, stream-table layout
    xt = np.zeros((NGRP * CORES * BSLOT, F), BF16)
    xt[grow_of_node] = (np.asarray(x, np.float32)
                        * dis[:, None]).astype(BF16)
    xt8 = xt.astype(FP8)

    # --- per-core edge data ---
    per_core = []
    cnt2 = np.zeros((CORES, NGRP * CH), np.int64)
    cnt1 = np.zeros((CORES, CH), np.int64)
    for c in range(CORES):
        m = core_of == c
        s_nodes = src[m]
        d_pos = pos_of_node[dst[m]]
        # append self-loop edges for the core's own nodes
        own = np.arange(c * NLOC, (c + 1) * NLOC)
        s1_nodes = np.concatenate([s_nodes, own])
        d1_pos = np.concatenate([d_pos, pos_of_node[own]])
        per_core.append((s_nodes, d_pos, s1_nodes, d1_pos))
        ch2 = d_pos >> 7
        sg2 = band_of_node[s_nodes]
        cnt2[c] = np.bincount(sg2 * CH + ch2, minlength=NGRP * CH)
        cnt1[c] = np.bincount(d1_pos >> 7, minlength=CH)

    pl2 = _piece_plan(cnt2.max(axis=0), NGRP * CH, CT2,
                      seg_grp=np.arange(NGRP * CH) // CH, ngrp=NGRP)
    pl1 = _piece_plan(cnt1.max(axis=0), CH, CT1,
                      seg_grp=np.arange(CH) // CHB, ngrp=NGRP)

    counts = np.bincount(batch, minlength=g_.G).astype(np.float32)
    invc = (1.0 / np.maximum(counts, 1.0)).astype(np.float32).reshape(g_.G, 1)

    in_maps = []
    for c in range(CORES):
        s_nodes, d_pos, s1_nodes, d1_pos = per_core[c]
        im = {}

        # ---- L2 streams: sort by (src band, dst chunk) ----
        sg2 = band_of_node[s_nodes]
        ch2 = d_pos >> 7
        sl2 = d_pos & 127
        seg2 = sg2 * CH + ch2
        o2 = np.argsort(seg2, kind='stable')
        rows2, seg2s, sl2s = row_of_node[s_nodes[o2]], seg2[o2], sl2[o2]
        seg_start = np.searchsorted(seg2s, np.arange(NGRP * CH))
        rank = np.arange(len(seg2s)) - seg_start[seg2s]
        pos2 = pl2["base"][seg2s] + rank
        idxv = np.zeros(pl2["S_total"], np.int16)
        idxv[pos2] = rows2.astype(np.int16)
        for g in range(NGRP):
            lo = pl2["grp_lo"][g]
            hi = lo + pl2["grp_tiles"][g] * 128
            seg16 = idxv[lo:hi].reshape(-1, 16).T
            im[f"idxg{g}"] = np.tile(seg16, (8, 1)).copy()
        e_g2 = seg2s // CH
        e_t2 = np.empty(len(pos2), np.int64)
        for g in range(NGRP):
            m2 = e_g2 == g
            e_t2[m2] = (pos2[m2] - pl2["grp_lo"][g]) // 128
        e_p2 = np.array([pl2["pk"][(g, t, sgm)] for g, t, sgm in
                         zip(e_g2, e_t2, seg2s)], np.int64)
        im["oh"] = _onehot_tiles(pl2["NP"], pos2, e_g2, pl2["grp_lo"],
                                 e_p2, sl2s)

        # ---- L1 stream (incl self-loops): sort by chunk ----
        ch1 = d1_pos >> 7
        sl1 = d1_pos & 127
        o1 = np.argsort(ch1, kind='stable')
        g1, ch1s, sl1s = grow_of_node[s1_nodes[o1]], ch1[o1], sl1[o1]
        seg_start1 = np.searchsorted(ch1s, np.arange(CH))
        rank1 = np.arange(len(ch1s)) - seg_start1[ch1s]
        pos1 = pl1["base"][ch1s] + rank1
        e_g1 = ch1s // CHB
        e_t1 = np.empty(len(pos1), np.int64)
        for g in range(NGRP):
            m1 = e_g1 == g
            e_t1[m1] = (pos1[m1] - pl1["grp_lo"][g]) // 128
        e_p1 = np.array([pl1["pk"][(g, t, sgm)] for g, t, sgm in
                         zip(e_g1, e_t1, ch1s)], np.int64)
        im["oh1"] = _onehot_tiles(pl1["NP"], pos1, e_g1,
                                  pl1["grp_lo"], e_p1, sl1s)
        stream_rows = np.zeros(pl1["S_total"], np.int64)
        stream_rows[pos1] = g1
        T1 = pl1["S_total"] // 128
        im["xs1"] = np.ascontiguousarray(
            xt8[stream_rows].reshape(T1, 128, F).transpose(1, 0, 2)
            .reshape(128, T1 * F))

        # ---- batch one-hot, dis columns, weights ----
        own = np.arange(c * NLOC, (c + 1) * NLOC)
        own_pos = pos_of_node[own]
        B = np.zeros(CH * 128 * g_.G, BF16)
        B[own_pos * g_.G + batch[own]] = np.float32(1.0)
        im["ohb"] = np.ascontiguousarray(
            B.reshape(CH, 128, g_.G).transpose(1, 0, 2).reshape(128, CH * g_.G))
        dcol = np.zeros(CH * 128, np.float32)
        dcol[own_pos] = dis[own]
        im["disc"] = np.ascontiguousarray(
            dcol.reshape(CH, 128).T.copy())      # [128, CH]
        im["invc"] = invc
        im["ident"] = np.eye(128, dtype=np.float32).astype(BF16)
        im["ident8"] = np.eye(128, dtype=np.float32).astype(FP8)
        im["w1"] = np.asarray(W1, np.float32).astype(BF16)
        im["w2"] = np.asarray(W2, np.float32).astype(BF16)
        im["fc1w"] = np.asarray(fc1_w, np.float32).astype(BF16)
        im["fc2w"] = np.asarray(fc2_w, np.float32).astype(BF16)
        im["b1"] = np.asarray(b1, np.float32).astype(BF16).reshape(1, F)
        im["b2"] = np.asarray(b2, np.float32).astype(BF16).reshape(1, F)
        im["fc2b"] = np.asarray(fc2_b, np.float32).astype(BF16).reshape(1, g_.A)
        im["fc1b"] = np.asarray(fc1_b, np.float32).reshape(F, 1).copy()
        in_maps.append(im)

    plan = dict(pl1=pl1, pl2=pl2,
                aux=dict(band_of_node=band_of_node, slot_of_node=slot_of_node,
                         row_of_node=row_of_node, dis=dis))
    return plan, in_maps


def _build(geom, plan, tag="", stages="all", nq=NQ):
    g_ = geom
    BSLOT, CH, GRP2, G, A = g_.BSLOT, g_.CH, g_.GRP2, g_.G, g_.A
    pl1, pl2 = plan["pl1"], plan["pl2"]
    T1 = pl1["S_total"] // 128
    bf = mybir.dt.bfloat16
    f8 = mybir.dt.float8e4
    f32 = mybir.dt.float32
    AL = mybir.AluOpType
    ACT = mybir.ActivationFunctionType

    nc = bacc.Bacc("TRN2", debug=False, target_bir_lowering=False,
                   num_swdge_queues=nq)
    P = {}
    def par(name, shape, dt):
        P[name] = nc.declare_dram_parameter(name + tag, list(shape), dt,
                                            isOutput=False)
        return P[name]

    for g in range(NGRP):
        par(f"idxg{g}", [128, pl2["grp_tiles"][g] * 8], mybir.dt.int16)
    par("oh", [128, pl2["NP"] * 128], f8)
    par("oh1", [128, pl1["NP"] * 128], f8)
    par("xs1", [128, T1 * F], f8)
    par("ohb", [128, CH * G], bf)
    par("disc", [128, CH], f32)
    par("invc", [G, 1], f32)
    par("ident", [128, 128], bf)
    par("ident8", [128, 128], f8)
    par("w1", [F, F], bf)
    par("w2", [F, F], bf)
    par("fc1w", [F, F], bf)
    par("fc2w", [F, A], bf)
    par("b1", [1, F], bf)
    par("b2", [1, F], bf)
    par("fc2b", [1, A], bf)
    par("fc1b", [F, 1], f32)
    out_ext = nc.declare_dram_parameter("out" + tag, [G, A], f32, isOutput=True)

    agin = [nc.dram_tensor(f"agin{b}" + tag, [BSLOT, F], bf)
            for b in range(NGRP)]
    tbl2 = [nc.dram_tensor(f"tbl2{b}" + tag, [GRP2, F], bf,
                           addr_space="Shared") for b in range(NGRP)]
    ar_in = nc.dram_tensor("arin" + tag, [G, F], f32)
    ar_out = nc.dram_tensor("arout" + tag, [G, F], f32, addr_space="Shared")

    with tile.TileContext(nc) as tc:
        with contextlib.ExitStack() as ex:
            pc = ex.enter_context(tc.tile_pool(name="const", bufs=1))
            pbig = ex.enter_context(tc.tile_pool(name="big", bufs=1))
            paccb = ex.enter_context(tc.tile_pool(name="accb", bufs=2))
            pidx = ex.enter_context(tc.tile_pool(name="idx", bufs=2))
            pxs = ex.enter_context(tc.tile_pool(name="xs", bufs=2))
            pgb = ex.enter_context(tc.tile_pool(name="gb", bufs=2))
            poh = ex.enter_context(tc.tile_pool(name="oh", bufs=2))
            ptf = ex.enter_context(tc.tile_pool(name="tf", bufs=4))
            pseg = ex.enter_context(tc.tile_pool(name="ps1", bufs=2,
                                                 space=bass.MemorySpace.PSUM))
            pseg2 = ex.enter_context(tc.tile_pool(name="ps2", bufs=2,
                                                  space=bass.MemorySpace.PSUM))
            ptp = ex.enter_context(tc.tile_pool(name="ptp", bufs=4,
                                                space=bass.MemorySpace.PSUM))

            nc.gpsimd.load_library(library_config.mlp)

            ct = {}
            for nm in ["ohb", "disc", "ident", "ident8", "w1", "w2",
                       "fc1w", "fc2w", "b1", "b2", "fc2b", "fc1b", "invc"]:
                t = pc.tile([P[nm].shape[0], P[nm].shape[1]], P[nm].dtype,
                            tag=nm)
                nc.sync.dma_start(out=t[:], in_=P[nm][:, :])
                ct[nm] = t
            ones = pc.tile([1, 128], bf)
            nc.vector.memset(ones[:], 1.0)

            rel1 = pbig.tile([128, CH * 128], bf)    # resident dis*relu1
            acc2 = pbig.tile([128, CH * 128], f32)   # L2 aggregation
            pacc = pbig.tile([G, F], f32)
            nc.vector.memset(acc2[:], 0.0)
            nc.vector.memset(pacc[:], 0.0)

            if stages == "noop":
                z0 = ptf.tile([G, A], f32)
                nc.vector.memset(z0[:], 0.0)
                nc.sync.dma_start(out=out_ext[:, :], in_=z0[:])
                nc.compile()
                return nc

            gq = [0]  # rotating SWDGE queue for gather calls

            # ---------- L2 group emission ----------
            l2_p_global = [0]
            l2_pos16 = [0] * NGRP
            ps2_hold = [None]

            def emit_l2_group(g):
                pos16 = 0
                t0call = 0
                for (ntiles, npieces) in pl2["call_plan"][g]:
                    nidx = ntiles * 128
                    idx_t = pidx.tile([128, nidx // 16], mybir.dt.int16)
                    nc.sync.dma_start(
                        out=idx_t[:],
                        in_=P[f"idxg{g}"][:, pos16:pos16 + nidx // 16])
                    gbuf = pgb.tile([128, ntiles, F], bf)
                    nc.gpsimd.dma_gather(
                        gbuf[:], tbl2[g].ap(),
                        idx_t[:], nidx, nidx, F, single_packet=False,
                        queue_num=gq[0] % nq)
                    gq[0] += 1
                    ohslab = poh.tile([128, max(npieces, 1), 128], f8,
                                      tag="oh2")
                    if npieces:
                        nc.sync.dma_start(
                            out=ohslab[:, :npieces, :],
                            in_=P["oh"].ap().rearrange(
                                "p (t d) -> p t d", d=128)[
                                :, l2_p_global[0]:l2_p_global[0] + npieces, :])
                    for pp in range(npieces):
                        p = l2_p_global[0] + pp
                        seg = int(pl2["piece_seg"][p])
                        chs = seg % CH
                        tloc = int(pl2["piece_tile"][p]) - t0call
                        if pl2["piece_first"][p]:
                            ps2_hold[0] = pseg2.tile([128, 128], f32,
                                                     name='ps2seg',
                                                     tag='ps2seg')
                        nc.tensor.matmul(ps2_hold[0][:], ohslab[:, pp, :],
                                         gbuf[:, tloc, :],
                                         start=bool(pl2["piece_first"][p]),
                                         stop=bool(pl2["piece_last"][p]))
                        if pl2["piece_last"][p]:
                            csl = acc2[:, chs * 128:(chs + 1) * 128]
                            nc.vector.tensor_tensor(csl, csl, ps2_hold[0][:],
                                                    AL.add)
                    l2_p_global[0] += npieces
                    t0call += ntiles
                    pos16 += nidx // 16

            # ---------- band pipeline ----------
            l1_p_global = 0
            ps_hold = [None]

            for b in range(NGRP):
                accband = paccb.tile([128, CHB * 128], f32, tag="accband")
                # L1 stream calls for this band (plan group b, band-aligned)
                t0c = 0
                tb0 = pl1["grp_lo"][b] // 128
                for (ntiles, npieces) in pl1["call_plan"][b]:
                    gbuf = pxs.tile([128, ntiles, F], f8)
                    nc.sync.dma_start(
                        out=gbuf[:],
                        in_=P["xs1"][:, (tb0 + t0c) * F:
                                     (tb0 + t0c + ntiles) * F])
                    ohslab = poh.tile([128, max(npieces, 1), 128], f8,
                                      tag="oh1")
                    if npieces:
                        nc.sync.dma_start(
                            out=ohslab[:, :npieces, :],
                            in_=P["oh1"].ap().rearrange(
                                "p (t d) -> p t d", d=128)[
                                :, l1_p_global:l1_p_global + npieces, :])
                    for pp in range(npieces):
                        p = l1_p_global + pp
                        chs = int(pl1["piece_seg"][p])
                        tloc = int(pl1["piece_tile"][p]) - t0c
                        if pl1["piece_first"][p]:
                            ps_hold[0] = pseg.tile([128, 128], f32,
                                                   name='ps1seg', tag='ps1seg')
                        nc.tensor.matmul(ps_hold[0][:], ohslab[:, pp, :],
                                         gbuf[:, tloc, :],
                                         start=bool(pl1["piece_first"][p]),
                                         stop=bool(pl1["piece_last"][p]))
                        if pl1["piece_last"][p]:
                            lch = chs - b * CHB
                            nc.vector.tensor_copy(
                                accband[:, lch * 128:(lch + 1) * 128],
                                ps_hold[0][:])
                    l1_p_global += npieces
                    t0c += ntiles

                # transforms for this band's chunks
                for lch in range(CHB):
                    ch = b * CHB + lch
                    aggS = ptf.tile([128, 128], bf, tag="aggS")
                    nc.vector.tensor_scalar(
                        aggS[:], accband[:, lch * 128:(lch + 1) * 128],
                        ct["disc"][:, ch:ch + 1], None, AL.mult)
                    psT = ptp.tile([128, 128], bf, tag="ps")
                    nc.tensor.transpose(psT[:], aggS[:], ct["ident"][:])
                    aggT = ptf.tile([128, 128], bf, tag="aggT")
                    nc.scalar.copy(aggT[:], psT[:])
                    psO = ptp.tile([128, 128], f32, tag="ps")
                    nc.tensor.matmul(psO[:], aggT[:], ct["w1"][:],
                                     start=True, stop=False)
                    nc.tensor.matmul(psO[:], ones[:1, :], ct["b1"][:1, :],
                                     start=False, stop=True)
                    rel_sl = rel1[:, ch * 128:(ch + 1) * 128]
                    nc.scalar.activation(rel_sl, psO[:], ACT.Relu,
                                         scale=ct["disc"][:, ch:ch + 1])
                    nc.sync.dma_start(
                        out=agin[b][lch * 128:(lch + 1) * 128, :],
                        in_=rel_sl)
                    # L2 self contribution: acc2[ch] += I @ rel1[ch]
                    psX = pseg2.tile([128, 128], f32, tag="ps2seg")
                    nc.tensor.matmul(psX[:], ct["ident8"][:], rel_sl,
                                     start=True, stop=True)
                    csl = acc2[:, ch * 128:(ch + 1) * 128]
                    nc.vector.tensor_tensor(csl, csl, psX[:], AL.add)

                nc.gpsimd.collective_compute(
                    "AllGather", AL.bypass,
                    ins=[agin[b].ap().opt()],
                    outs=[tbl2[b].ap().opt()],
                    replica_groups=[list(range(CORES))])
                if stages == "all":
                    emit_l2_group(b)

            # ---------- phase 3: L2 transform + pooling ----------
            for ch in (range(CH) if stages == "all" else range(0)):
                aggS = ptf.tile([128, 128], bf, tag="aggS")
                nc.vector.tensor_scalar(
                    aggS[:], acc2[:, ch * 128:(ch + 1) * 128],
                    ct["disc"][:, ch:ch + 1], None, AL.mult)
                psT = ptp.tile([128, 128], bf, tag="ps")
                nc.tensor.transpose(psT[:], aggS[:], ct["ident"][:])
                aggT = ptf.tile([128, 128], bf, tag="aggT")
                nc.scalar.copy(aggT[:], psT[:])
                psO = ptp.tile([128, 128], f32, tag="ps")
                nc.tensor.matmul(psO[:], aggT[:], ct["w2"][:],
                                 start=True, stop=False)
                nc.tensor.matmul(psO[:], ones[:1, :], ct["b2"][:1, :],
                                 start=False, stop=True)
                rel_t = ptf.tile([128, 128], bf, tag="rel")
                nc.scalar.activation(rel_t[:], psO[:], ACT.Relu)
                psB = ptp.tile([G, F], f32, tag="ps")
                nc.tensor.matmul(psB[:], ct["ohb"][:, ch * G:(ch + 1) * G],
                                 rel_t[:], start=True, stop=True)
                nc.vector.tensor_tensor(pacc[:], pacc[:], psB[:], AL.add)

            # ---------- phase 4: AllReduce + MLP head ----------
            if stages != "all":
                z0 = ptf.tile([G, A], f32)
                nc.vector.memset(z0[:], 0.0)
                nc.sync.dma_start(out=out_ext[:, :], in_=z0[:])
                nc.compile()
                return nc
            nc.sync.dma_start(out=ar_in[:, :], in_=pacc[:])
            nc.gpsimd.collective_compute(
                "AllReduce", AL.add,
                ins=[ar_in.ap().opt()], outs=[ar_out.ap().opt()],
                replica_groups=[list(range(CORES))])
            pooledf = ptf.tile([G, F], f32)
            nc.sync.dma_start(out=pooledf[:], in_=ar_out[:, :])
            pooledb = ptf.tile([G, F], bf)
            nc.vector.tensor_scalar(pooledb[:], pooledf[:], ct["invc"][:, :1],
                                    None, AL.mult)
            psPT = ptp.tile([F, G], bf, tag="ps")
            nc.tensor.transpose(psPT[:], pooledb[:], ct["ident"][:G, :G])
            pooledT = ptf.tile([F, G], bf)
            nc.scalar.copy(pooledT[:], psPT[:])
            psZ = ptp.tile([F, G], f32, tag="ps")
            nc.tensor.matmul(psZ[:], ct["fc1w"][:], pooledT[:],
                             start=True, stop=True)
            zT = ptf.tile([F, G], bf)
            nc.scalar.activation(zT[:], psZ[:], ACT.Relu, bias=ct["fc1b"][:, :1])
            psO2 = ptp.tile([G, A], f32, tag="ps")
            nc.tensor.matmul(psO2[:], zT[:], ct["fc2w"][:],
                             start=True, stop=False)
            nc.tensor.matmul(psO2[:], ones[:1, :G], ct["fc2b"][:1, :],
                             start=False, stop=True)
            outt = ptf.tile([G, A], f32)
            nc.scalar.activation(outt[:], psO2[:], ACT.Sigmoid)
            nc.sync.dma_start(out=out_ext[:, :], in_=outt[:])

    nc.compile()
    return nc


_GEOM = Geom()
_CALLS = [0]


def kernel(x, edge_index, batch, W1, b1, W2, b2, fc1_w, fc1_b, fc2_w, fc2_b):
    plan, in_maps = _prep(_GEOM, x, edge_index, batch, W1, b1, W2, b2,
                          fc1_w, fc1_b, fc2_w, fc2_b)
    tag = f"_k{_CALLS[0]}" if _CALLS[0] else ""
    _CALLS[0] += 1
    nc = _build(_GEOM, plan, tag=tag)
    res = run_bass_kernel_spmd(nc, [{k + tag: v for k, v in m.items()}
                                    for m in in_maps],
                               list(range(CORES)))
    return np.asarray(res.results[0]["out" if not tag else "out" + tag],
                      dtype=np.float32)
